# revision 15
# baseline (speedup 1.0000x reference)
"""Trainium2 Bass kernel for nn_AutoEncIndex_33887291965861 (topk_masking).

Reference computation:
    soft  = softmax((mat + noise) / temperature)            [training w/ gumbel]
    index = top_k(soft, J).indices                          (full descending sort)
    sel   = greedy row-by-row assignment (first J rows pick the best
            still-unused joint; later rows pick their argmax)
    out   = stop_grad(one_hot(sel)) - stop_grad(mat) + mat

Key facts used here:
  * (0 - m) + m == +0.0 exactly in IEEE fp32, so the output is an exact
    one-hot matrix except the selected entry is (1 - m) + m which is within
    1-2 ulp of 1.0.  Emitting exactly 1.0 keeps the total relative error
    at ~2e-7.
  * softmax and /temperature are strictly monotone per row, so the selection
    order is the order of w = mat + noise (fp32), with lowest-index
    tie-breaking (lax.top_k semantics == vector-engine max_index semantics).
  * The greedy pass over the first J rows selects, for row r, the
    still-available joint with the highest w[r] value (proof: the first
    available joint in row r's descending order always sits within the
    first r+1 positions by pigeonhole, which is exactly the cols<=r window
    the reference uses).  Rows >= J just take their argmax.

Device kernel (SPMD over 8 cores, row-sharded, 4096 rows/core):
  stream 2 MB chunks (512 rows laid out as 128 partitions x 4 row segments),
  w = mat + noise on the vector engine, per-segment argmax via max/max_index,
  one-hot built on the scalar engine as Relu(1 - |iota - idx|), stream out.
  Memory bound: 48 MB of HBM traffic per core.

Host: the inherently-sequential greedy over the first 1024 rows (tiny), then
patch those rows of the gathered output.
"""

import os

import numpy as np

HW = 32768
J = 1024
N_CORES = 8
ROWS_PER_CORE = HW // N_CORES  # 4096
P = 128  # SBUF partitions

_NC_CACHE = {}


def _build_nc(rows_per_core: int, j: int, r: int, onehot_engine: str = "act",
              repeat: int = 1):
    """Build the per-core Bass module.

    Input "mn" is [2, rows_per_core, j] fp32 — mat stacked with noise (one
    tensor so each chunk loads with a single DMA instruction / single
    semaphore: TRN2 compute instructions can carry only one sync wait).
    Output "out" is the exact one-hot of the per-row argmax of mat + noise.
    r = rows per partition per chunk (chunk covers 128*r rows).
    """
    import concourse.bacc as bacc
    import concourse.mybir as mybir
    from concourse.tile import TileContext

    chunk_rows = P * r
    assert rows_per_core % chunk_rows == 0, (rows_per_core, chunk_rows)
    n_chunks = rows_per_core // chunk_rows
    f32 = mybir.dt.float32

    # Bacc (not raw Bass): its finalize() runs generate_event_semaphores,
    # which splits multi-sem waits — TRN2 instructions carry at most one.
    nc = bacc.Bacc()
    mn = nc.dram_tensor("mn", [2, rows_per_core, j], f32, kind="ExternalInput")
    out = nc.dram_tensor("out", [rows_per_core, j], f32, kind="ExternalOutput")

    # chunk c, partition p holds rows (c*128 + p)*r .. +r-1, contiguous in DRAM
    mnv = mn[:, :, :].rearrange("t (c p r) m -> c p t (r m)", p=P, r=r)
    outv = out[:, :].rearrange("(c p r) m -> c p (r m)", p=P, r=r)

    with TileContext(nc) as tc:
        with (
            tc.tile_pool(name="const", bufs=1) as cpool,
            tc.tile_pool(name="work", bufs=2) as pool,
            tc.tile_pool(name="small", bufs=3) as spool,
        ):
            iota_i = cpool.tile([P, j], mybir.dt.int32)
            nc.gpsimd.iota(iota_i[:], [[1, j]], channel_multiplier=0)
            iota_f = cpool.tile([P, j], f32)
            nc.vector.tensor_copy(iota_f[:], iota_i[:])

            for c in [c for _ in range(repeat) for c in range(n_chunks)]:
                tmn = pool.tile([P, 2, r * j], f32, tag="mn")
                nc.sync.dma_start(tmn[:, :, :], mnv[c])
                w = pool.tile([P, r * j], f32, tag="w")
                nc.vector.tensor_add(w[:], tmn[:, 0, :], tmn[:, 1, :])
                ot = pool.tile([P, r * j], f32, tag="out")
                mx = spool.tile([P, 8 * r], f32, tag="mx")
                ix = spool.tile([P, 8 * r], mybir.dt.uint32, tag="ix")
                for s in range(r):
                    seg = w[:, s * j : (s + 1) * j]
                    oseg = ot[:, s * j : (s + 1) * j]
                    nc.vector.max(mx[:, 8 * s : 8 * s + 8], seg)
                    nc.vector.max_index(ix[:, 8 * s : 8 * s + 8], mx[:, 8 * s : 8 * s + 8], seg)
                    if onehot_engine == "act":
                        # one-hot on the scalar engine: Relu(1 - |iota - idx|)
                        ixn = spool.tile([P, 1], f32, tag="ixn")
                        nc.vector.tensor_scalar_mul(ixn[:], ix[:, 8 * s : 8 * s + 1], -1.0)
                        ab = spool.tile([P, j], f32, tag="abs")
                        nc.scalar.activation(
                            ab[:], iota_f[:], mybir.ActivationFunctionType.Abs,
                            bias=ixn[:], scale=1.0,
                        )
                        nc.scalar.activation(
                            oseg, ab[:], mybir.ActivationFunctionType.Relu,
                            bias=1.0, scale=-1.0,
                        )
                    else:
                        # one-hot on the vector engine: (iota == idx)
                        nc.vector.tensor_scalar(
                            oseg, iota_i[:], ix[:, 8 * s : 8 * s + 1], None,
                            op0=mybir.AluOpType.is_equal,
                        )
                nc.sync.dma_start(outv[c], ot[:])
    nc.finalize()
    return nc


def _get_nc(rows_per_core=ROWS_PER_CORE, j=J, r=4, onehot_engine=None, repeat=1):
    if onehot_engine is None:
        onehot_engine = os.environ.get("KERNEL_ONEHOT", "act")
    key = (rows_per_core, j, r, onehot_engine, repeat)
    if key not in _NC_CACHE:
        _NC_CACHE[key] = _build_nc(*key)
    return _NC_CACHE[key]


def _greedy_select(w_first: np.ndarray) -> np.ndarray:
    """Sequential greedy: row r takes the available joint with max w[r].

    Equivalent to the reference's scan over descending top-k indices.
    """
    n = w_first.shape[0]
    avail = np.ones(n, dtype=bool)
    sel = np.empty(n, dtype=np.int64)
    neg_inf = np.float32(-np.inf)
    for r in range(n):
        row = np.where(avail, w_first[r], neg_inf)
        s = int(np.argmax(row))
        sel[r] = s
        avail[s] = False
    return sel


_RUNNER_CACHE = {}


def _make_runner(r: int = 4, onehot_engine=None, repeat: int = 1):
    """Cached runner around run_bass_kernel_spmd.

    The first call goes through run_bass_kernel_spmd (the supported axon/PJRT
    path); during it we capture the jitted SPMD callable that
    run_bass_via_pjrt builds internally, so subsequent calls (and timing
    loops) reuse the compiled executable instead of re-tracing/re-compiling
    (run_bass_via_pjrt creates a fresh jit closure per invocation).
    """
    key = (r, onehot_engine, repeat)
    if key in _RUNNER_CACHE:
        return _RUNNER_CACHE[key]

    import jax
    from concourse.bass_utils import run_bass_kernel_spmd

    nc = _get_nc(ROWS_PER_CORE, J, r, onehot_engine, repeat)
    state = {"fn": None}

    def runner(mn_global: np.ndarray) -> np.ndarray:
        """mn_global: (2*N_CORES, ROWS_PER_CORE, J) per-core [mat, noise]
        pairs. Returns (HW, J) output."""
        if state["fn"] is None:
            in_maps = [{"mn": mn_global[2 * k : 2 * k + 2]} for k in range(N_CORES)]
            orig_jit = jax.jit

            def capturing_jit(f, *a, **kw):
                g = orig_jit(f, *a, **kw)
                if "donate_argnums" in kw and kw.get("keep_unused"):
                    state["fn"] = g
                return g

            jax.jit = capturing_jit
            try:
                res = run_bass_kernel_spmd(nc, in_maps, core_ids=list(range(N_CORES)))
            finally:
                jax.jit = orig_jit
            return np.concatenate([r_["out"] for r_ in res.results], axis=0)
        outs = state["fn"](mn_global, np.zeros((HW, J), np.float32))
        out = outs[0] if isinstance(outs, (tuple, list)) else outs
        return np.asarray(out)

    runner.state = state
    _RUNNER_CACHE[key] = runner
    return runner


def stack_inputs(mat: np.ndarray, noise: np.ndarray) -> np.ndarray:
    """Global (2*N_CORES, ROWS_PER_CORE, J): per-core [mat_shard, noise_shard]
    pairs along axis 0, so a P("core") shard is exactly the NEFF's (2, rows, J)
    "mn" tensor."""
    m3 = mat.reshape(N_CORES, ROWS_PER_CORE, J)
    n3 = noise.reshape(N_CORES, ROWS_PER_CORE, J)
    return np.stack([m3, n3], axis=1).reshape(2 * N_CORES, ROWS_PER_CORE, J)


def run_device(mat: np.ndarray, noise: np.ndarray, r: int = 4, onehot_engine=None):
    """Shard row-wise over 8 cores, run the Bass kernel, gather."""
    runner = _make_runner(r, onehot_engine)
    out = runner(stack_inputs(mat, noise))
    return np.asarray(out)


def kernel(sgt_trans_mat, gumbel_noise, use_gumbel_noise=1, is_training=1,
           temperature=30):
    mat = np.ascontiguousarray(np.asarray(sgt_trans_mat, dtype=np.float32))
    assert mat.shape == (HW, J), mat.shape
    training = bool(int(np.asarray(is_training)))
    use_g = training and bool(int(np.asarray(use_gumbel_noise)))
    if use_g:
        noise = np.ascontiguousarray(np.asarray(gumbel_noise, dtype=np.float32))
    else:
        # selection order falls back to mat itself; temperature never matters
        noise = np.zeros_like(mat)

    out = run_device(mat, noise)

    # Host-side greedy over the first J rows (inherently sequential, tiny),
    # then patch those rows of the output.
    w_first = mat[:J] + noise[:J]  # same IEEE fp32 add as the device
    sel = _greedy_select(w_first)
    out[:J] = 0.0
    out[np.arange(J), sel] = np.float32(1.0)
    return out


# revision 29
# speedup vs baseline: 3.3562x; 3.3562x over previous
"""Trainium2 Bass kernel for nn_AutoEncIndex_33887291965861 (topk_masking).

Reference computation:
    soft  = softmax((mat + noise) / temperature)            [training w/ gumbel]
    index = top_k(soft, J).indices                          (full descending sort)
    sel   = greedy row-by-row assignment (first J rows pick the best
            still-unused joint; later rows pick their argmax)
    out   = stop_grad(one_hot(sel)) - stop_grad(mat) + mat

Key facts used here:
  * (0 - m) + m == +0.0 exactly in IEEE fp32, so the output is an exact
    one-hot matrix except the selected entry is (1 - m) + m which is within
    1-2 ulp of 1.0.  Emitting exactly 1.0 keeps the total relative error
    at ~2e-7.
  * softmax and /temperature are strictly monotone per row, so the selection
    order is the order of w = mat + noise (fp32), with lowest-index
    tie-breaking (lax.top_k semantics == vector-engine max_index semantics).
  * The greedy pass over the first J rows selects, for row r, the
    still-available joint with the highest w[r] value (proof: the first
    available joint in row r's descending order always sits within the
    first r+1 positions by pigeonhole, which is exactly the cols<=r window
    the reference uses).  Rows >= J just take their argmax.

Device kernel (SPMD over 8 cores, row-sharded, 4096 rows/core):
  stream 2 MB chunks (512 rows laid out as 128 partitions x 4 row segments),
  w = mat + noise on the vector engine, per-segment argmax via max/max_index,
  one-hot built on the scalar engine as Relu(1 - |iota - idx|), stream out.
  Memory bound: 48 MB of HBM traffic per core.

Host: the inherently-sequential greedy over the first 1024 rows (tiny), then
patch those rows of the gathered output.
"""

import os

import numpy as np

HW = 32768
J = 1024
N_CORES = 8
ROWS_PER_CORE = HW // N_CORES  # 4096
P = 128  # SBUF partitions

_NC_CACHE = {}


def _build_nc(rows_per_core: int, j: int, r: int, onehot_engine: str = "act",
              repeat: int = 1, mode: str = "full", bufs: int = 2,
              out_engine: str = "sync", out_dt: str = "u8"):
    """Build the per-core Bass module.

    Input "mn" is [2, rows_per_core, j] fp32 — mat stacked with noise (one
    tensor so each chunk loads with a single DMA instruction / single
    semaphore: TRN2 compute instructions can carry only one sync wait).
    Output "out" is the exact one-hot of the per-row argmax of mat + noise.
    r = rows per partition per chunk (chunk covers 128*r rows).
    """
    import concourse.bacc as bacc
    import concourse.mybir as mybir
    from concourse.tile import TileContext

    chunk_rows = P * r
    assert rows_per_core % chunk_rows == 0, (rows_per_core, chunk_rows)
    n_chunks = rows_per_core // chunk_rows
    f32 = mybir.dt.float32

    # Bacc (not raw Bass): its finalize() runs generate_event_semaphores,
    # which splits multi-sem waits — TRN2 instructions carry at most one.
    nc = bacc.Bacc()
    odt = {"f32": f32, "u8": mybir.dt.uint8, "bf16": mybir.dt.bfloat16}[out_dt]
    mn = nc.dram_tensor("mn", [2, rows_per_core, j], f32, kind="ExternalInput")
    out = nc.dram_tensor("out", [rows_per_core, j], odt, kind="ExternalOutput")

    # chunk c, partition p holds rows (c*128 + p)*r .. +r-1, contiguous in DRAM
    mnv = mn[:, :, :].rearrange("t (c p r) m -> c p t (r m)", p=P, r=r)
    outv = out[:, :].rearrange("(c p r) m -> c p (r m)", p=P, r=r)

    out_dma = {"sync": nc.sync, "scalar": nc.scalar, "gpsimd": nc.gpsimd}[out_engine]
    with TileContext(nc) as tc:
        with (
            tc.tile_pool(name="const", bufs=1) as cpool,
            tc.tile_pool(name="work", bufs=bufs) as pool,
            tc.tile_pool(name="small", bufs=3) as spool,
        ):
            iota_i = cpool.tile([P, j], mybir.dt.int32)
            nc.gpsimd.iota(iota_i[:], [[1, j]], channel_multiplier=0)
            iota_f = cpool.tile([P, j], f32)
            nc.vector.tensor_copy(iota_f[:], iota_i[:])

            for c in [c for _ in range(repeat) for c in range(n_chunks)]:
                tmn = pool.tile([P, 2, r * j], f32, tag="mn")
                if mode in ("split2", "loadonly2"):
                    # mat half on the SP HWDGE ring, noise half on the ACT ring
                    nc.sync.dma_start(tmn[:, 0, :], mnv[c][:, 0, :])
                    nc.scalar.dma_start(tmn[:, 1, :], mnv[c][:, 1, :])
                else:
                    nc.sync.dma_start(tmn[:, :, :], mnv[c])
                if mode in ("loadonly", "loadonly2"):
                    continue
                if mode == "dmaonly":
                    ot = pool.tile([P, r * j], odt, tag="out")
                    nc.vector.tensor_copy(ot[:], tmn[:, 0, :])
                    out_dma.dma_start(outv[c], ot[:])
                    continue
                w = pool.tile([P, r * j], f32, tag="w")
                nc.vector.tensor_add(w[:], tmn[:, 0, :], tmn[:, 1, :])
                ot = pool.tile([P, r * j], odt, tag="out")
                mx = spool.tile([P, 8 * r], f32, tag="mx")
                ix = spool.tile([P, 8 * r], mybir.dt.uint32, tag="ix")
                for s in range(r):
                    seg = w[:, s * j : (s + 1) * j]
                    oseg = ot[:, s * j : (s + 1) * j]
                    nc.vector.max(mx[:, 8 * s : 8 * s + 8], seg)
                    nc.vector.max_index(ix[:, 8 * s : 8 * s + 8], mx[:, 8 * s : 8 * s + 8], seg)
                    if onehot_engine == "act":
                        # one-hot on the scalar engine: Relu(1 - |iota - idx|)
                        ixn = spool.tile([P, 1], f32, tag="ixn")
                        nc.vector.tensor_scalar_mul(ixn[:], ix[:, 8 * s : 8 * s + 1], -1.0)
                        ab = spool.tile([P, j], f32, tag="abs")
                        nc.scalar.activation(
                            ab[:], iota_f[:], mybir.ActivationFunctionType.Abs,
                            bias=ixn[:], scale=1.0,
                        )
                        nc.scalar.activation(
                            oseg, ab[:], mybir.ActivationFunctionType.Relu,
                            bias=1.0, scale=-1.0,
                        )
                    else:
                        # one-hot on the vector engine: (iota == idx), f32 compare
                        ixf = spool.tile([P, 1], f32, tag="ixf")
                        nc.vector.tensor_scalar_mul(ixf[:], ix[:, 8 * s : 8 * s + 1], 1.0)
                        nc.vector.tensor_scalar(
                            oseg, iota_f[:], ixf[:], None,
                            op0=mybir.AluOpType.is_equal,
                        )
                out_dma.dma_start(outv[c], ot[:])
    nc.finalize()
    return nc


def _get_nc(rows_per_core=ROWS_PER_CORE, j=J, r=4, onehot_engine=None, repeat=1,
            mode="full", bufs=2, out_engine="sync", out_dt=None):
    if onehot_engine is None:
        onehot_engine = os.environ.get("KERNEL_ONEHOT", "act")
    if out_dt is None:
        out_dt = os.environ.get("KERNEL_OUT_DT", "u8")
    key = (rows_per_core, j, r, onehot_engine, repeat, mode, bufs, out_engine, out_dt)
    if key not in _NC_CACHE:
        _NC_CACHE[key] = _build_nc(*key)
    return _NC_CACHE[key]


def _greedy_select(w_first: np.ndarray) -> np.ndarray:
    """Sequential greedy: row r takes the available joint with max w[r].

    Equivalent to the reference's scan over descending top-k indices.
    """
    n = w_first.shape[0]
    avail = np.ones(n, dtype=bool)
    sel = np.empty(n, dtype=np.int64)
    neg_inf = np.float32(-np.inf)
    for r in range(n):
        row = np.where(avail, w_first[r], neg_inf)
        s = int(np.argmax(row))
        sel[r] = s
        avail[s] = False
    return sel


_RUNNER_CACHE = {}


def _make_runner(r: int = 4, onehot_engine=None, repeat: int = 1, mode: str = "full",
                 bufs: int = 2, out_engine: str = "sync", out_dt=None):
    """Cached runner around run_bass_kernel_spmd.

    The first call goes through run_bass_kernel_spmd (the supported axon/PJRT
    path); during it we capture the jitted SPMD callable that
    run_bass_via_pjrt builds internally, so subsequent calls (and timing
    loops) reuse the compiled executable instead of re-tracing/re-compiling
    (run_bass_via_pjrt creates a fresh jit closure per invocation).
    """
    key = (r, onehot_engine, repeat, mode, bufs, out_engine, out_dt)
    if key in _RUNNER_CACHE:
        return _RUNNER_CACHE[key]

    import jax
    from concourse.bass_utils import run_bass_kernel_spmd

    nc = _get_nc(ROWS_PER_CORE, J, r, onehot_engine, repeat, mode, bufs, out_engine,
                 out_dt)
    state = {"fn": None}

    def runner(mn_global: np.ndarray) -> np.ndarray:
        """mn_global: (2*N_CORES, ROWS_PER_CORE, J) per-core [mat, noise]
        pairs. Returns (HW, J) output."""
        if state["fn"] is None:
            in_maps = [{"mn": mn_global[2 * k : 2 * k + 2]} for k in range(N_CORES)]
            orig_jit = jax.jit

            def capturing_jit(f, *a, **kw):
                g = orig_jit(f, *a, **kw)
                if "donate_argnums" in kw and kw.get("keep_unused"):
                    state["fn"] = g
                return g

            jax.jit = capturing_jit
            try:
                res = run_bass_kernel_spmd(nc, in_maps, core_ids=list(range(N_CORES)))
            finally:
                jax.jit = orig_jit
            out = np.concatenate([r_["out"] for r_ in res.results], axis=0)
            state["out_np_dtype"] = out.dtype
            return out
        outs = state["fn"](mn_global, np.zeros((HW, J), state["out_np_dtype"]))
        out = outs[0] if isinstance(outs, (tuple, list)) else outs
        return np.asarray(out)

    runner.state = state
    _RUNNER_CACHE[key] = runner
    return runner


def stack_inputs(mat: np.ndarray, noise: np.ndarray) -> np.ndarray:
    """Global (2*N_CORES, ROWS_PER_CORE, J): per-core [mat_shard, noise_shard]
    pairs along axis 0, so a P("core") shard is exactly the NEFF's (2, rows, J)
    "mn" tensor."""
    m3 = mat.reshape(N_CORES, ROWS_PER_CORE, J)
    n3 = noise.reshape(N_CORES, ROWS_PER_CORE, J)
    return np.stack([m3, n3], axis=1).reshape(2 * N_CORES, ROWS_PER_CORE, J)


def run_device(mat: np.ndarray, noise: np.ndarray, r: int = 4, onehot_engine=None):
    """Shard row-wise over 8 cores, run the Bass kernel, gather."""
    runner = _make_runner(r, onehot_engine)
    out = runner(stack_inputs(mat, noise))
    return np.asarray(out)


def kernel(sgt_trans_mat, gumbel_noise, use_gumbel_noise=1, is_training=1,
           temperature=30):
    mat = np.ascontiguousarray(np.asarray(sgt_trans_mat, dtype=np.float32))
    assert mat.shape == (HW, J), mat.shape
    training = bool(int(np.asarray(is_training)))
    use_g = training and bool(int(np.asarray(use_gumbel_noise)))
    if use_g:
        noise = np.ascontiguousarray(np.asarray(gumbel_noise, dtype=np.float32))
    else:
        # selection order falls back to mat itself; temperature never matters
        noise = np.zeros_like(mat)

    out = run_device(mat, noise)
    # device output may be uint8/bf16 (exact for one-hot); return float32
    if out.dtype != np.float32:
        out = out.astype(np.float32)
    elif not out.flags.writeable:
        out = out.copy()

    # Host-side greedy over the first J rows (inherently sequential, tiny),
    # then patch those rows of the output.
    w_first = mat[:J] + noise[:J]  # same IEEE fp32 add as the device
    sel = _greedy_select(w_first)
    out[:J] = 0.0
    out[np.arange(J), sel] = np.float32(1.0)
    return out
